# revision 2
# baseline (speedup 1.0000x reference)
import os
import sys

import numpy as np

sys.path.insert(0, "/opt/trn_rl_repo")

import ml_dtypes
import concourse.bass as bass
from concourse import bacc
import concourse.mybir as mybir
import concourse.tile as tile
from concourse.bass_utils import run_bass_kernel_spmd

# Problem constants (hardcoded per contract)
B, L, N, H, HU = 512, 16, 10000, 128, 128
NCORES = 8
BL = B // NCORES            # 64 local batch rows per core
T2 = 2 * L                  # 32 node/coord time steps
COLS = T2 * BL              # 2048 columns, t-major: col = t*BL + b
LCOLS = L * BL              # 1024 tau columns
KT = 128
NKT = 80                    # k-tiles (padded: 80*128 = 10240 >= 10000)
NPAD = NKT * KT
KCH = 4                     # k-tiles per DMA chunk (2 MB chunks)
NCH = NKT // KCH

F32 = mybir.dt.float32
BF16 = mybir.dt.bfloat16
NPBF = ml_dtypes.bfloat16

SIG = mybir.ActivationFunctionType.Sigmoid
TANH = mybir.ActivationFunctionType.Tanh
IDENT = mybir.ActivationFunctionType.Identity

ADD = mybir.AluOpType.add
MUL = mybir.AluOpType.mult

# bf16 packed constants (matmul operands), column offsets
C_WC = 0
C_WTAU = 128
C_WX2 = 256
C_WRES = 384
C_WE2 = 512
C_WX1 = 640
C_WE1 = 641
C_W2 = 642            # [128, 7]
C_XIN = 649           # [128, 64]
C_T0 = 713
C_END = 777
C_TAU = 841           # [128, 1024]
C_COORDS = 1865       # [128, 2048]
CPW = 3920

# fp32 packed biases, column offsets
Z_BTAU = 0
Z_BX2 = 1
Z_BRES = 2
Z_BE2 = 3
Z_B1 = 8              # [128, 7]
Z_B2 = 15             # [1, 7] (row 0)
CBW = 24

# Phase structure: all-active LSTMs batched per superstep, prefix-ordered by
# descending sequence length so the active set is always a prefix.
P1 = [6, 3, 1]        # T = [51, 32, 19]; non-node LSTMs (overlap node GEMM)
P2 = [0, 5, 4, 2]     # T = [83, 64, 51, 32]; node-dependent LSTMs
TLEN = {0: 5 * L + 3, 1: L + 3, 2: 2 * L, 3: 2 * L, 4: 3 * L + 3,
        5: 4 * L, 6: 3 * L + 3}
W1 = len(P1) * BL     # 192 gate-block width, phase 1
W2 = len(P2) * BL     # 256 gate-block width, phase 2
# PSUM gate block order [i, f, o, g] -> torch gate index (i, f, g, o)
TORCH = [0, 1, 3, 2]

_prog_cache = {}


def _build_program():
    """One SPMD Bass program; every core runs it on its own 64-row batch shard."""
    nc = bacc.Bacc()

    d_x = nc.declare_dram_parameter("xk", [NCH, 128, KCH, COLS], BF16, isOutput=False)
    d_wn = nc.declare_dram_parameter("wn", [128, NKT, H], BF16, isOutput=False)
    d_cp = nc.declare_dram_parameter("cpack", [128, CPW], BF16, isOutput=False)
    d_cb = nc.declare_dram_parameter("cbias", [128, CBW], F32, isOutput=False)
    d_bg1 = nc.declare_dram_parameter("bg1", [128, 4 * W1], F32, isOutput=False)
    d_bg2 = nc.declare_dram_parameter("bg2", [128, 4 * W2], F32, isOutput=False)
    d_wih = nc.declare_dram_parameter("wihT", [H, 7, 4 * H], BF16, isOutput=False)
    d_whh = nc.declare_dram_parameter("whhT", [H, 7, 4 * H], BF16, isOutput=False)
    d_w1 = nc.declare_dram_parameter("w1T", [H, 7, HU], BF16, isOutput=False)
    d_out = nc.declare_dram_parameter("out", [1, 7 * BL], F32, isOutput=True)

    with tile.TileContext(nc) as tc:
        with (
            tc.tile_pool(name="consts", bufs=1) as consts,
            tc.tile_pool(name="xpool", bufs=3) as xpool,
            tc.tile_pool(name="gact", bufs=2) as gactp,
            tc.tile_pool(name="stp", bufs=2) as stp,
        ):
            cp = consts.tile([128, CPW], BF16, tag="cp")
            nc.sync.dma_start(cp[:], d_cp[:])
            cb = consts.tile([128, CBW], F32, tag="cb")
            nc.sync.dma_start(cb[:], d_cb[:])
            bg1 = consts.tile([128, 4, W1], F32, tag="bg1")
            nc.sync.dma_start(bg1[:], d_bg1[:])
            bg2 = consts.tile([128, 4, W2], F32, tag="bg2")
            nc.sync.dma_start(bg2[:], d_bg2[:])
            wih_sb = consts.tile([H, 7, 4 * H], BF16, tag="wih")
            nc.sync.dma_start(wih_sb[:], d_wih[:])
            whh_sb = consts.tile([H, 7, 4 * H], BF16, tag="whh")
            nc.sync.dma_start(whh_sb[:], d_whh[:])
            w1_sb = consts.tile([H, 7, HU], BF16, tag="w1")
            nc.sync.dma_start(w1_sb[:], d_w1[:])
            wn_sb = consts.tile([128, NKT, H], BF16, tag="wn")
            nc.sync.dma_start(wn_sb[:], d_wn[:])

            tauh_sb = consts.tile([H, LCOLS], BF16, tag="tauh")
            coordh_sb = consts.tile([H, COLS], BF16, tag="coordh")
            nodeh_sb = consts.tile([H, COLS], BF16, tag="nodeh")
            xh_sb = consts.tile([H, BL], BF16, tag="xh")
            t0h_sb = consts.tile([H, BL], BF16, tag="t0h")
            endh_sb = consts.tile([H, BL], BF16, tag="endh")
            out_sb = consts.tile([1, 7 * BL], F32, tag="outsb")

            # persistent LSTM states (prefix-ordered column blocks per phase)
            h1 = consts.tile([H, W1], BF16, tag="h1")
            c1 = consts.tile([H, W1], BF16, tag="c1")
            h2 = consts.tile([H, W2], BF16, tag="h2")
            c2 = consts.tile([H, W2], BF16, tag="c2")

            # ---- small projections ----
            with tc.tile_pool(name="ps1", bufs=2, space="PSUM") as ps1:
                for j in range(LCOLS // 512):
                    ps = ps1.tile([128, 512], F32, tag="ps")
                    nc.tensor.matmul(ps[:], cp[:, C_WTAU:C_WTAU + 128],
                                     cp[:, C_TAU + j * 512:C_TAU + (j + 1) * 512],
                                     start=True, stop=True)
                    nc.scalar.activation(tauh_sb[:, j * 512:(j + 1) * 512], ps[:],
                                         TANH, bias=cb[:, Z_BTAU:Z_BTAU + 1])

                ps1_t = ps1.tile([128, 512], F32, tag="ps", name="psx1")
                psa = ps1_t[0:1, 0:BL]
                nc.tensor.matmul(psa[:], cp[:, C_WX1:C_WX1 + 1],
                                 cp[:, C_XIN:C_XIN + BL], start=True, stop=True)
                s1_sb = stp.tile([128, BL], BF16, tag="svec")
                nc.vector.memset(s1_sb[:], 0.0)
                nc.vector.tensor_copy(s1_sb[0:1, :], psa[:])
                ps2_t = ps1.tile([128, 512], F32, tag="ps", name="psx2")
                psb = ps2_t[:, 0:BL]
                nc.tensor.matmul(psb[:], cp[:, C_WX2:C_WX2 + 128], s1_sb[:],
                                 start=True, stop=True)
                nc.scalar.activation(xh_sb[:], psb[:], TANH,
                                     bias=cb[:, Z_BX2:Z_BX2 + 1])

                ps3_t = ps1.tile([128, 512], F32, tag="ps", name="psx3")
                psc = ps3_t[:, 0:BL]
                nc.tensor.matmul(psc[:], cp[:, C_WRES:C_WRES + 128],
                                 cp[:, C_T0:C_T0 + BL], start=True, stop=True)
                nc.scalar.activation(t0h_sb[:], psc[:], TANH,
                                     bias=cb[:, Z_BRES:Z_BRES + 1])

                ps4_t = ps1.tile([128, 512], F32, tag="ps", name="psx4")
                psd = ps4_t[0:1, 0:BL]
                nc.tensor.matmul(psd[:], cp[:, C_WE1:C_WE1 + 1],
                                 cp[:, C_END:C_END + BL], start=True, stop=True)
                s2_sb = stp.tile([128, BL], BF16, tag="svec")
                nc.vector.memset(s2_sb[:], 0.0)
                nc.vector.tensor_copy(s2_sb[0:1, :], psd[:])
                ps5_t = ps1.tile([128, 512], F32, tag="ps", name="psx5")
                pse = ps5_t[:, 0:BL]
                nc.tensor.matmul(pse[:], cp[:, C_WE2:C_WE2 + 128], s2_sb[:],
                                 start=True, stop=True)
                nc.scalar.activation(endh_sb[:], pse[:], IDENT,
                                     bias=cb[:, Z_BE2:Z_BE2 + 1])

                for j in range(COLS // 512):
                    ps = ps1.tile([128, 512], F32, tag="ps")
                    nc.tensor.matmul(ps[:], cp[:, C_WC:C_WC + 128],
                                     cp[:, C_COORDS + j * 512:C_COORDS + (j + 1) * 512],
                                     start=True, stop=True)
                    nc.vector.tensor_copy(coordh_sb[:, j * 512:(j + 1) * 512], ps[:])

            # ---- sequences ----
            def ncol(t):
                return nodeh_sb[:, t * BL:(t + 1) * BL]

            def ccol(t):
                return coordh_sb[:, t * BL:(t + 1) * BL]

            def lcol(l):
                return tauh_sb[:, l * BL:(l + 1) * BL]

            seqs = {}
            pre = [lambda: xh_sb[:], lambda: t0h_sb[:]]
            suf = [lambda: endh_sb[:]]
            mk = lambda f, *a: (lambda: f(*a))
            seqs[0] = pre + [mk(f, t) for l in range(L)
                             for f, t in ((lcol, l), (ncol, 2 * l), (ccol, 2 * l),
                                          (ncol, 2 * l + 1), (ccol, 2 * l + 1))] + suf
            seqs[1] = pre + [mk(lcol, l) for l in range(L)] + suf
            seqs[2] = [mk(ncol, t) for t in range(T2)]
            seqs[3] = [mk(ccol, t) for t in range(T2)]
            seqs[4] = pre + [mk(f, t) for l in range(L)
                             for f, t in ((lcol, l), (ncol, 2 * l),
                                          (ncol, 2 * l + 1))] + suf
            seqs[5] = [mk(f, t) for l in range(L)
                       for f, t in ((ncol, 2 * l), (ccol, 2 * l),
                                    (ncol, 2 * l + 1), (ccol, 2 * l + 1))]
            seqs[6] = pre + [mk(f, t) for l in range(L)
                             for f, t in ((lcol, l), (ccol, 2 * l),
                                          (ccol, 2 * l + 1))] + suf

            def head(k, h_ap, pool):
                hp_t = pool.tile([128, 512], F32, tag="gp", name=f"hp{k}")
                hp = hp_t[:, 0:BL]
                nc.tensor.matmul(hp[:], w1_sb[:, k], h_ap, start=True, stop=True)
                z1 = stp.tile([128, BL], BF16, tag="z1")
                nc.scalar.activation(z1[:], hp[:], TANH,
                                     bias=cb[:, Z_B1 + k:Z_B1 + k + 1])
                op_t = pool.tile([128, 512], F32, tag="gp", name=f"op{k}")
                op = op_t[0:1, 0:BL]
                nc.tensor.matmul(op[:], cp[:, C_W2 + k:C_W2 + k + 1], z1[:],
                                 start=True, stop=True)
                nc.scalar.activation(out_sb[:, k * BL:(k + 1) * BL], op[:],
                                     IDENT, bias=cb[0:1, Z_B2 + k:Z_B2 + k + 1])

            def superstep(t, ks, Wp, gpool, bias_sb, h_t, c_t):
                """One batched LSTM step for the active prefix of ks."""
                A = sum(1 for k in ks if t < TLEN[k])
                cols = A * BL
                gp = gpool.tile([128, 4, Wp], F32, tag="gp")
                for p in range(4):
                    tg = TORCH[p]
                    gs = slice(tg * H, (tg + 1) * H)
                    for j in range(A):
                        k = ks[j]
                        xc = seqs[k][t]()
                        dst = gp[:, p, j * BL:(j + 1) * BL]
                        if t == 0:
                            nc.tensor.matmul(dst, wih_sb[:, k, gs], xc,
                                             start=True, stop=True)
                        else:
                            # ih(start)+hh(accumulate): pairs must stay adjacent
                            # (start=True clears has_written bank-wide).
                            nc.tensor.matmul(dst, wih_sb[:, k, gs], xc,
                                             start=True, stop=False)
                            nc.tensor.matmul(dst, whh_sb[:, k, gs],
                                             h_t[:, j * BL:(j + 1) * BL],
                                             start=False, stop=True)
                ga = gactp.tile([128, 4, Wp], BF16, tag="gact")
                nc.vector.tensor_tensor(ga[:, :, :cols], gp[:, :, :cols],
                                        bias_sb[:, :, :cols], ADD)
                nc.scalar.activation(ga[:, 0:3, :cols], ga[:, 0:3, :cols], SIG)
                nc.scalar.activation(ga[:, 3, :cols], ga[:, 3, :cols], TANH)
                if t == 0:
                    nc.vector.tensor_tensor(c_t[:, :cols], ga[:, 0, :cols],
                                            ga[:, 3, :cols], MUL)
                else:
                    ig = stp.tile([128, Wp], BF16, tag="ig")
                    nc.vector.tensor_tensor(ig[:, :cols], ga[:, 0, :cols],
                                            ga[:, 3, :cols], MUL)
                    fc = stp.tile([128, Wp], BF16, tag="fc")
                    nc.vector.tensor_tensor(fc[:, :cols], ga[:, 1, :cols],
                                            c_t[:, :cols], MUL)
                    nc.vector.tensor_tensor(c_t[:, :cols], fc[:, :cols],
                                            ig[:, :cols], ADD)
                tcc = stp.tile([128, Wp], BF16, tag="tanhc")
                nc.scalar.activation(tcc[:, :cols], c_t[:, :cols], TANH)
                nc.vector.tensor_tensor(h_t[:, :cols], ga[:, 2, :cols],
                                        tcc[:, :cols], MUL)

            # ---- node GEMM (streams under phase 1) ----
            with tc.tile_pool(name="psum_gemm", bufs=1, space="PSUM") as psum_gemm:
                gps = [psum_gemm.tile([128, 512], F32, tag=f"gemm{j}",
                                      name=f"gemm{j}") for j in range(4)]
                for ch in range(NCH):
                    xt = xpool.tile([128, KCH, COLS], BF16, tag="xt")
                    nc.sync.dma_start(xt[:], d_x[ch])
                    for kt in range(KCH):
                        kk = ch * KCH + kt
                        for j in range(4):
                            nc.tensor.matmul(gps[j][:], wn_sb[:, kk],
                                             xt[:, kt, j * 512:(j + 1) * 512],
                                             start=(kk == 0), stop=(kk == NKT - 1))
                for j in range(4):
                    nc.vector.tensor_copy(nodeh_sb[:, j * 512:(j + 1) * 512],
                                          gps[j][:])

                # Phase 1: non-node LSTMs overlap the GEMM DMA stream
                with tc.tile_pool(name="gp1", bufs=2, space="PSUM") as gp1:
                    for t in range(TLEN[P1[0]]):
                        superstep(t, P1, W1, gp1, bg1, h1, c1)
                    for j, k in enumerate(P1):
                        head(k, h1[:, j * BL:(j + 1) * BL], gp1)

            # Phase 2: node-dependent LSTMs
            with tc.tile_pool(name="gp2", bufs=2, space="PSUM") as gp2:
                for t in range(TLEN[P2[0]]):
                    superstep(t, P2, W2, gp2, bg2, h2, c2)
                for j, k in enumerate(P2):
                    head(k, h2[:, j * BL:(j + 1) * BL], gp2)

            nc.sync.dma_start(d_out[:], out_sb[:])

    nc.finalize()
    return nc


def _get_program():
    if "nc" not in _prog_cache:
        _prog_cache["nc"] = _build_program()
    return _prog_cache["nc"]


def _pack_constants(inp):
    cpk = np.zeros((128, CPW), NPBF)
    cbk = np.zeros((128, CBW), np.float32)

    def put(dst, c, arr):
        dst[:arr.shape[0], c:c + arr.shape[1]] = arr

    put(cpk, C_WC, inp["Wcoord"].T)
    put(cpk, C_WTAU, inp["Wtau"].T)
    put(cpk, C_WX2, inp["Wx2"].T)
    put(cpk, C_WRES, inp["Wres"].T)
    put(cpk, C_WE2, inp["Wend2"].T)
    put(cpk, C_WX1, inp["Wx1"].T)
    put(cpk, C_WE1, inp["Wend1"].T)
    put(cpk, C_W2, inp["head_W2"].reshape(7, HU).T)
    put(cbk, Z_BTAU, inp["btau"][:, None])
    put(cbk, Z_BX2, inp["bx2"][:, None])
    put(cbk, Z_BRES, inp["bres"][:, None])
    put(cbk, Z_BE2, inp["bend2"][:, None])
    put(cbk, Z_B1, inp["head_b1"].T)
    put(cbk, Z_B2, inp["head_b2"].reshape(1, 7))

    bsum = (inp["lstm_bih"] + inp["lstm_bhh"]).reshape(7, 4, H)
    bg1 = np.zeros((128, 4, W1), np.float32)
    bg2 = np.zeros((128, 4, W2), np.float32)
    for dst, ks in ((bg1, P1), (bg2, P2)):
        for p in range(4):
            tg = TORCH[p]
            for j, k in enumerate(ks):
                dst[:, p, j * BL:(j + 1) * BL] = bsum[k, tg][:, None]
    return cpk, cbk, bg1.reshape(128, 4 * W1), bg2.reshape(128, 4 * W2)


def _make_in_maps(inp):
    node = inp["node_inputs"]
    coords = inp["coords"]
    tau = inp["tau_inputs"]
    x = inp["x"]
    t0 = inp["t0_res"]
    end = inp["end"]

    wn = np.zeros((NPAD, H), NPBF)
    wn[:N] = inp["Wnode"].T
    wn_dev = np.ascontiguousarray(wn.reshape(NKT, 128, H).transpose(1, 0, 2))

    wih = np.ascontiguousarray(inp["lstm_Wih"].transpose(2, 0, 1).astype(NPBF))
    whh = np.ascontiguousarray(inp["lstm_Whh"].transpose(2, 0, 1).astype(NPBF))
    w1 = np.ascontiguousarray(inp["head_W1"].transpose(2, 0, 1).astype(NPBF))

    cpk_base, cbk, bg1, bg2 = _pack_constants(inp)

    in_maps = []
    for c in range(NCORES):
        sl = slice(c * BL, (c + 1) * BL)
        xk = np.zeros((NPAD, COLS), NPBF)
        xk[:N] = node[sl].transpose(2, 1, 0).reshape(N, COLS)
        xk_dev = np.ascontiguousarray(
            xk.reshape(NCH, KCH, 128, COLS).transpose(0, 2, 1, 3))
        cpk = cpk_base.copy()
        cpk[:2, C_XIN:C_XIN + BL] = x[sl].T
        cpk[:1, C_T0:C_T0 + BL] = t0[sl].T
        cpk[:2, C_END:C_END + BL] = end[sl].T
        cpk[:1, C_TAU:C_TAU + LCOLS] = tau[sl].transpose(2, 1, 0).reshape(1, LCOLS)
        cpk[:2, C_COORDS:C_COORDS + COLS] = coords[sl].transpose(2, 1, 0).reshape(2, COLS)
        in_maps.append(dict(
            xk=xk_dev, wn=wn_dev, cpack=cpk, cbias=cbk,
            bg1=bg1, bg2=bg2, wihT=wih, whhT=whh, w1T=w1,
        ))
    return in_maps


def kernel(**inputs):
    inp = {k: np.asarray(v, dtype=np.float32) for k, v in inputs.items()}
    in_maps = _make_in_maps(inp)
    nc = _get_program()
    res = run_bass_kernel_spmd(nc, in_maps, core_ids=list(range(NCORES)))
    if res.exec_time_ns is not None:
        print(f"HW exec time: {res.exec_time_ns} ns")

    outs = [r["out"].reshape(7, BL) for r in res.results]
    full = np.concatenate(outs, axis=1)      # [7, B]
    return tuple(full[k][:, None].astype(np.float32) for k in range(7))


# revision 3
# speedup vs baseline: 1.5942x; 1.5942x over previous
import os
import sys

import numpy as np

sys.path.insert(0, "/opt/trn_rl_repo")

import ml_dtypes
import concourse.bass as bass
from concourse import bacc
import concourse.mybir as mybir
import concourse.tile as tile
from concourse.bass_utils import run_bass_kernel_spmd

# Problem constants (hardcoded per contract)
B, L, N, H, HU = 512, 16, 10000, 128, 128
NCORES = 8
BL = B // NCORES            # 64 local batch rows per core
T2 = 2 * L                  # 32 node/coord time steps
COLS = T2 * BL              # 2048 node/coord columns, t-major: col = t*BL + b
LCOLS = L * BL              # 1024 tau columns
KT = 128
NKT = 80                    # k-tiles (padded: 80*128 = 10240 >= 10000)
NPAD = NKT * KT
NCB = 4                     # nodeh column blocks (512 cols each)
CBC = 5                     # DMA chunks per column block (16 k-tiles, 2 MB each)
KCH = NKT // CBC            # 16 k-tiles per chunk

F32 = mybir.dt.float32
BF16 = mybir.dt.bfloat16
NPBF = ml_dtypes.bfloat16

SIG = mybir.ActivationFunctionType.Sigmoid
TANH = mybir.ActivationFunctionType.Tanh
IDENT = mybir.ActivationFunctionType.Identity

ADD = mybir.AluOpType.add
MUL = mybir.AluOpType.mult

# bf16 packed constants (matmul operands), column offsets
C_WC = 0
C_WTAU = 128
C_WX2 = 256
C_WRES = 384
C_WE2 = 512
C_WX1 = 640
C_WE1 = 641
C_W2 = 642            # [128, 7]
C_XIN = 649           # [128, 64]
C_T0 = 713
C_END = 777
C_TAU = 841           # [128, 1024]
C_COORDS = 1865       # [128, 2048]
CPW = 3920

# fp32 packed scalar biases, column offsets
Z_BTAU = 0
Z_BX2 = 1
Z_BRES = 2
Z_BE2 = 3
Z_B1 = 8              # [128, 7]
Z_B2 = 15             # [1, 7] (row 0)
CBW = 24

# Independent LSTM group chains, each prefix-ordered by descending T so the
# active set is always a prefix.  Gate block order [i, f, o, g].
GROUPS = [[0], [5], [4, 6], [2, 3, 1]]
TLEN = {0: 5 * L + 3, 1: L + 3, 2: 2 * L, 3: 2 * L, 4: 3 * L + 3,
        5: 4 * L, 6: 3 * L + 3}
GW = [len(g) * BL for g in GROUPS]          # 64, 64, 128, 192
TORCH = [0, 1, 3, 2]                        # block p -> torch gate index
NGK = sum(4 * len(g) for g in GROUPS)       # one-hot rows total (28)
BOH_OFF = []
_off = 0
for _g in GROUPS:
    BOH_OFF.append(_off)
    _off += 4 * len(_g) * BL
BOHW = _off                                  # 1792

_prog_cache = {}


def _build_program():
    """One SPMD Bass program; every core runs it on its own 64-row batch shard."""
    nc = bacc.Bacc()

    d_x = nc.declare_dram_parameter("xk", [NCB, CBC, 128, KCH, 512], BF16,
                                    isOutput=False)
    d_wn = nc.declare_dram_parameter("wn", [128, NKT, H], BF16, isOutput=False)
    d_cp = nc.declare_dram_parameter("cpack", [128, CPW], BF16, isOutput=False)
    d_cb = nc.declare_dram_parameter("cbias", [128, CBW], F32, isOutput=False)
    d_bt = nc.declare_dram_parameter("biasT", [NGK, 4 * 128], BF16, isOutput=False)
    d_oh = nc.declare_dram_parameter("onehot", [NGK, BOHW], BF16, isOutput=False)
    d_wih = nc.declare_dram_parameter("wihT", [H, 7, 4 * H], BF16, isOutput=False)
    d_whh = nc.declare_dram_parameter("whhT", [H, 7, 4 * H], BF16, isOutput=False)
    d_w1 = nc.declare_dram_parameter("w1T", [H, 7, HU], BF16, isOutput=False)
    d_out = nc.declare_dram_parameter("out", [1, 7 * BL], F32, isOutput=True)

    with tile.TileContext(nc) as tc:
        with (
            tc.tile_pool(name="consts", bufs=1) as consts,
            tc.tile_pool(name="xpool", bufs=3) as xpool,
            tc.tile_pool(name="gact", bufs=2) as gactp,
            tc.tile_pool(name="stp", bufs=2) as stp,
            tc.tile_pool(name="ps1", bufs=1, space="PSUM") as ps1,
            tc.tile_pool(name="gemm", bufs=2, space="PSUM") as gemmp,
            tc.tile_pool(name="g0", bufs=1, space="PSUM") as g0p,
            tc.tile_pool(name="g1", bufs=1, space="PSUM") as g1p,
            tc.tile_pool(name="g2", bufs=1, space="PSUM") as g2p,
            tc.tile_pool(name="g3", bufs=1, space="PSUM") as g3p,
        ):
            gpools = [g0p, g1p, g2p, g3p]

            cp = consts.tile([128, CPW], BF16, tag="cp")
            nc.sync.dma_start(cp[:], d_cp[:])
            cb = consts.tile([128, CBW], F32, tag="cb")
            nc.sync.dma_start(cb[:], d_cb[:])
            bt = consts.tile([NGK, 4 * 128], BF16, tag="bt")
            nc.sync.dma_start(bt[:], d_bt[:])
            oh = consts.tile([NGK, BOHW], BF16, tag="oh")
            nc.sync.dma_start(oh[:], d_oh[:])
            wih_sb = consts.tile([H, 7, 4 * H], BF16, tag="wih")
            nc.sync.dma_start(wih_sb[:], d_wih[:])
            whh_sb = consts.tile([H, 7, 4 * H], BF16, tag="whh")
            nc.sync.dma_start(whh_sb[:], d_whh[:])
            w1_sb = consts.tile([H, 7, HU], BF16, tag="w1")
            nc.sync.dma_start(w1_sb[:], d_w1[:])
            wn_sb = consts.tile([128, NKT, H], BF16, tag="wn")
            nc.sync.dma_start(wn_sb[:], d_wn[:])

            tauh_sb = consts.tile([H, LCOLS], BF16, tag="tauh")
            coordh_sb = consts.tile([H, COLS], BF16, tag="coordh")
            nodeh_cb = [consts.tile([H, 512], BF16, tag=f"nodeh{c}",
                                    name=f"nodeh{c}") for c in range(NCB)]
            xh_sb = consts.tile([H, BL], BF16, tag="xh")
            t0h_sb = consts.tile([H, BL], BF16, tag="t0h")
            endh_sb = consts.tile([H, BL], BF16, tag="endh")
            out_sb = consts.tile([1, 7 * BL], F32, tag="outsb")

            # persistent LSTM states per group (bf16, prefix column blocks)
            h_t = [consts.tile([H, w], BF16, tag=f"h{g}", name=f"h{g}")
                   for g, w in enumerate(GW)]
            c_t = [consts.tile([H, w], BF16, tag=f"c{g}", name=f"c{g}")
                   for g, w in enumerate(GW)]

            # ---- small projections ----
            for j in range(LCOLS // 512):
                ps = ps1.tile([128, 512], F32, tag="ps")
                nc.tensor.matmul(ps[:], cp[:, C_WTAU:C_WTAU + 128],
                                 cp[:, C_TAU + j * 512:C_TAU + (j + 1) * 512],
                                 start=True, stop=True)
                nc.scalar.activation(tauh_sb[:, j * 512:(j + 1) * 512], ps[:],
                                     TANH, bias=cb[:, Z_BTAU:Z_BTAU + 1])

            pst = ps1.tile([128, 512], F32, tag="ps", name="psx1")
            psa = pst[0:1, 0:BL]
            nc.tensor.matmul(psa[:], cp[:, C_WX1:C_WX1 + 1],
                             cp[:, C_XIN:C_XIN + BL], start=True, stop=True)
            s1_sb = stp.tile([128, BL], BF16, tag="svec")
            nc.vector.memset(s1_sb[:], 0.0)
            nc.vector.tensor_copy(s1_sb[0:1, :], psa[:])
            pst = ps1.tile([128, 512], F32, tag="ps", name="psx2")
            psb = pst[:, 0:BL]
            nc.tensor.matmul(psb[:], cp[:, C_WX2:C_WX2 + 128], s1_sb[:],
                             start=True, stop=True)
            nc.scalar.activation(xh_sb[:], psb[:], TANH,
                                 bias=cb[:, Z_BX2:Z_BX2 + 1])

            pst = ps1.tile([128, 512], F32, tag="ps", name="psx3")
            psc = pst[:, 0:BL]
            nc.tensor.matmul(psc[:], cp[:, C_WRES:C_WRES + 128],
                             cp[:, C_T0:C_T0 + BL], start=True, stop=True)
            nc.scalar.activation(t0h_sb[:], psc[:], TANH,
                                 bias=cb[:, Z_BRES:Z_BRES + 1])

            pst = ps1.tile([128, 512], F32, tag="ps", name="psx4")
            psd = pst[0:1, 0:BL]
            nc.tensor.matmul(psd[:], cp[:, C_WE1:C_WE1 + 1],
                             cp[:, C_END:C_END + BL], start=True, stop=True)
            s2_sb = stp.tile([128, BL], BF16, tag="svec")
            nc.vector.memset(s2_sb[:], 0.0)
            nc.vector.tensor_copy(s2_sb[0:1, :], psd[:])
            pst = ps1.tile([128, 512], F32, tag="ps", name="psx5")
            pse = pst[:, 0:BL]
            nc.tensor.matmul(pse[:], cp[:, C_WE2:C_WE2 + 128], s2_sb[:],
                             start=True, stop=True)
            nc.scalar.activation(endh_sb[:], pse[:], IDENT,
                                 bias=cb[:, Z_BE2:Z_BE2 + 1])

            for j in range(COLS // 512):
                ps = ps1.tile([128, 512], F32, tag="ps")
                nc.tensor.matmul(ps[:], cp[:, C_WC:C_WC + 128],
                                 cp[:, C_COORDS + j * 512:C_COORDS + (j + 1) * 512],
                                 start=True, stop=True)
                nc.vector.tensor_copy(coordh_sb[:, j * 512:(j + 1) * 512], ps[:])

            # ---- node GEMM: column-block streamed so nodeh lands early ----
            for cbk in range(NCB):
                gps = gemmp.tile([128, 512], F32, tag="gemmps")
                for ch in range(CBC):
                    xt = xpool.tile([128, KCH, 512], BF16, tag="xt")
                    nc.sync.dma_start(xt[:], d_x[cbk, ch])
                    for kt in range(KCH):
                        kk = ch * KCH + kt
                        nc.tensor.matmul(gps[:], wn_sb[:, kk], xt[:, kt],
                                         start=(kk == 0), stop=(kk == NKT - 1))
                nc.vector.tensor_copy(nodeh_cb[cbk][:], gps[:])

            # ---- sequences ----
            def ncol(t):
                return nodeh_cb[t >> 3][:, (t & 7) * BL:((t & 7) + 1) * BL]

            def ccol(t):
                return coordh_sb[:, t * BL:(t + 1) * BL]

            def lcol(l):
                return tauh_sb[:, l * BL:(l + 1) * BL]

            seqs = {}
            pre = [lambda: xh_sb[:], lambda: t0h_sb[:]]
            suf = [lambda: endh_sb[:]]
            mk = lambda f, *a: (lambda: f(*a))
            seqs[0] = pre + [mk(f, t) for l in range(L)
                             for f, t in ((lcol, l), (ncol, 2 * l), (ccol, 2 * l),
                                          (ncol, 2 * l + 1), (ccol, 2 * l + 1))] + suf
            seqs[1] = pre + [mk(lcol, l) for l in range(L)] + suf
            seqs[2] = [mk(ncol, t) for t in range(T2)]
            seqs[3] = [mk(ccol, t) for t in range(T2)]
            seqs[4] = pre + [mk(f, t) for l in range(L)
                             for f, t in ((lcol, l), (ncol, 2 * l),
                                          (ncol, 2 * l + 1))] + suf
            seqs[5] = [mk(f, t) for l in range(L)
                       for f, t in ((ncol, 2 * l), (ccol, 2 * l),
                                    (ncol, 2 * l + 1), (ccol, 2 * l + 1))]
            seqs[6] = pre + [mk(f, t) for l in range(L)
                             for f, t in ((lcol, l), (ccol, 2 * l),
                                          (ccol, 2 * l + 1))] + suf

            def superstep(g, t):
                ks = GROUPS[g]
                W = GW[g]
                A = sum(1 for k in ks if t < TLEN[k])
                cols = A * BL
                nk = len(ks)
                gp = gpools[g].tile([128, 4, W], F32, tag="gp")
                flat = gp[:].rearrange("p a b -> p (a b)")
                # bias matmul(s) first: start=True covers each whole bank, so
                # all ih/hh matmuls accumulate with start=False afterwards.
                lhs = bt[0:4 * nk, g * 128:(g + 1) * 128]
                nbank = (4 * W + 511) // 512
                for b in range(nbank):
                    lo, hi = b * 512, min((b + 1) * 512, 4 * W)
                    nc.tensor.matmul(flat[:, lo:hi], lhs,
                                     oh[0:4 * nk, BOH_OFF[g] + lo:BOH_OFF[g] + hi],
                                     start=True, stop=False, skip_group_check=True)
                for p in range(4):
                    tg = TORCH[p]
                    gsl = slice(tg * H, (tg + 1) * H)
                    for j in range(A):
                        k = ks[j]
                        xc = seqs[k][t]()
                        dst = gp[:, p, j * BL:(j + 1) * BL]
                        last = (p == 3 and j == A - 1)
                        nc.tensor.matmul(dst, wih_sb[:, k, gsl], xc,
                                         start=False, stop=last and t == 0,
                                         skip_group_check=True)
                        if t > 0:
                            nc.tensor.matmul(dst, whh_sb[:, k, gsl],
                                             h_t[g][:, j * BL:(j + 1) * BL],
                                             start=False, stop=last,
                                             skip_group_check=True)
                ga = gactp.tile([128, 4, W], BF16, tag=f"gact{g}")
                nc.scalar.activation(ga[:, 0:3, :cols], gp[:, 0:3, :cols], SIG)
                nc.scalar.activation(ga[:, 3, :cols], gp[:, 3, :cols], TANH)
                if t == 0:
                    nc.vector.tensor_tensor(c_t[g][:, :cols], ga[:, 0, :cols],
                                            ga[:, 3, :cols], MUL)
                else:
                    ig = stp.tile([128, W], BF16, tag=f"ig{g}")
                    nc.vector.tensor_tensor(ig[:, :cols], ga[:, 0, :cols],
                                            ga[:, 3, :cols], MUL)
                    fc = stp.tile([128, W], BF16, tag=f"fc{g}")
                    nc.vector.tensor_tensor(fc[:, :cols], ga[:, 1, :cols],
                                            c_t[g][:, :cols], MUL)
                    nc.vector.tensor_tensor(c_t[g][:, :cols], fc[:, :cols],
                                            ig[:, :cols], ADD)
                tcc = stp.tile([128, W], BF16, tag=f"tc{g}")
                nc.scalar.activation(tcc[:, :cols], c_t[g][:, :cols], TANH)
                nc.vector.tensor_tensor(h_t[g][:, :cols], ga[:, 2, :cols],
                                        tcc[:, :cols], MUL)

            # interleave the group chains in program order per timestep
            tmax = max(TLEN[g[0]] for g in GROUPS)
            for t in range(tmax):
                for g in range(len(GROUPS)):
                    if t < TLEN[GROUPS[g][0]]:
                        superstep(g, t)

            # ---- heads ----
            def head(k, h_ap, pool):
                hp_t = pool.tile([128, 512], F32, tag="gp", name=f"hp{k}")
                hp = hp_t[:, 0:BL]
                nc.tensor.matmul(hp[:], w1_sb[:, k], h_ap, start=True, stop=True)
                z1 = stp.tile([128, BL], BF16, tag="z1")
                nc.scalar.activation(z1[:], hp[:], TANH,
                                     bias=cb[:, Z_B1 + k:Z_B1 + k + 1])
                op_t = pool.tile([128, 512], F32, tag="gp", name=f"op{k}")
                op = op_t[0:1, 0:BL]
                nc.tensor.matmul(op[:], cp[:, C_W2 + k:C_W2 + k + 1], z1[:],
                                 start=True, stop=True)
                nc.scalar.activation(out_sb[:, k * BL:(k + 1) * BL], op[:],
                                     IDENT, bias=cb[0:1, Z_B2 + k:Z_B2 + k + 1])

            for g, ks in enumerate(GROUPS):
                for j, k in enumerate(ks):
                    head(k, h_t[g][:, j * BL:(j + 1) * BL], gpools[g])

            nc.sync.dma_start(d_out[:], out_sb[:])

    nc.finalize()
    return nc


def _get_program():
    if "nc" not in _prog_cache:
        _prog_cache["nc"] = _build_program()
    return _prog_cache["nc"]


def _pack_constants(inp):
    cpk = np.zeros((128, CPW), NPBF)
    cbk = np.zeros((128, CBW), np.float32)

    def put(dst, c, arr):
        dst[:arr.shape[0], c:c + arr.shape[1]] = arr

    put(cpk, C_WC, inp["Wcoord"].T)
    put(cpk, C_WTAU, inp["Wtau"].T)
    put(cpk, C_WX2, inp["Wx2"].T)
    put(cpk, C_WRES, inp["Wres"].T)
    put(cpk, C_WE2, inp["Wend2"].T)
    put(cpk, C_WX1, inp["Wx1"].T)
    put(cpk, C_WE1, inp["Wend1"].T)
    put(cpk, C_W2, inp["head_W2"].reshape(7, HU).T)
    put(cbk, Z_BTAU, inp["btau"][:, None])
    put(cbk, Z_BX2, inp["bx2"][:, None])
    put(cbk, Z_BRES, inp["bres"][:, None])
    put(cbk, Z_BE2, inp["bend2"][:, None])
    put(cbk, Z_B1, inp["head_b1"].T)
    put(cbk, Z_B2, inp["head_b2"].reshape(1, 7))

    # bias-matmul constants: lhsT rows r = p*nk + j hold bsum[k_j, torch(p)];
    # one-hot rhs puts that row's 128-vector into block column (p, j)
    bsum = (inp["lstm_bih"] + inp["lstm_bhh"]).reshape(7, 4, H)
    btk = np.zeros((NGK, 4 * 128), np.float32)
    ohk = np.zeros((NGK, BOHW), NPBF)
    for g, ks in enumerate(GROUPS):
        nk = len(ks)
        for p in range(4):
            tg = TORCH[p]
            for j, k in enumerate(ks):
                r = p * nk + j
                btk[r, g * 128:(g + 1) * 128] = bsum[k, tg]
                base = BOH_OFF[g] + (p * nk + j) * BL
                ohk[r, base:base + BL] = 1.0
    return cpk, cbk, btk.astype(NPBF), ohk


def _make_in_maps(inp):
    node = inp["node_inputs"]
    coords = inp["coords"]
    tau = inp["tau_inputs"]
    x = inp["x"]
    t0 = inp["t0_res"]
    end = inp["end"]

    wn = np.zeros((NPAD, H), NPBF)
    wn[:N] = inp["Wnode"].T
    wn_dev = np.ascontiguousarray(wn.reshape(NKT, 128, H).transpose(1, 0, 2))

    wih = np.ascontiguousarray(inp["lstm_Wih"].transpose(2, 0, 1).astype(NPBF))
    whh = np.ascontiguousarray(inp["lstm_Whh"].transpose(2, 0, 1).astype(NPBF))
    w1 = np.ascontiguousarray(inp["head_W1"].transpose(2, 0, 1).astype(NPBF))

    cpk_base, cbk, btk, ohk = _pack_constants(inp)

    in_maps = []
    for c in range(NCORES):
        sl = slice(c * BL, (c + 1) * BL)
        xk = np.zeros((NPAD, COLS), NPBF)
        xk[:N] = node[sl].transpose(2, 1, 0).reshape(N, COLS)
        # [NCB, CBC, 128, KCH, 512]: col-block-major streaming layout
        xk_dev = np.ascontiguousarray(
            xk.reshape(CBC, KCH, 128, NCB, 512).transpose(3, 0, 2, 1, 4))
        cpk = cpk_base.copy()
        cpk[:2, C_XIN:C_XIN + BL] = x[sl].T
        cpk[:1, C_T0:C_T0 + BL] = t0[sl].T
        cpk[:2, C_END:C_END + BL] = end[sl].T
        cpk[:1, C_TAU:C_TAU + LCOLS] = tau[sl].transpose(2, 1, 0).reshape(1, LCOLS)
        cpk[:2, C_COORDS:C_COORDS + COLS] = coords[sl].transpose(2, 1, 0).reshape(2, COLS)
        in_maps.append(dict(
            xk=xk_dev, wn=wn_dev, cpack=cpk, cbias=cbk,
            biasT=btk, onehot=ohk, wihT=wih, whhT=whh, w1T=w1,
        ))
    return in_maps


def kernel(**inputs):
    inp = {k: np.asarray(v, dtype=np.float32) for k, v in inputs.items()}
    in_maps = _make_in_maps(inp)
    nc = _get_program()
    res = run_bass_kernel_spmd(nc, in_maps, core_ids=list(range(NCORES)))
    if res.exec_time_ns is not None:
        print(f"HW exec time: {res.exec_time_ns} ns")

    outs = [r["out"].reshape(7, BL) for r in res.results]
    full = np.concatenate(outs, axis=1)      # [7, B]
    return tuple(full[k][:, None].astype(np.float32) for k in range(7))


# revision 6
# speedup vs baseline: 1.6372x; 1.0270x over previous
import os
import sys

import numpy as np

sys.path.insert(0, "/opt/trn_rl_repo")

import ml_dtypes
import concourse.bass as bass
from concourse import bacc
import concourse.mybir as mybir
import concourse.tile as tile
from concourse.bass_utils import run_bass_kernel_spmd

# Problem constants (hardcoded per contract)
B, L, N, H, HU = 512, 16, 10000, 128, 128
NCORES = 8
BL = B // NCORES            # 64 local batch rows per core
T2 = 2 * L                  # 32 node/coord time steps
COLS = T2 * BL              # 2048 node/coord columns, t-major: col = t*BL + b
LCOLS = L * BL              # 1024 tau columns
KT = 128
NKT = 80                    # k-tiles (padded: 80*128 = 10240 >= 10000)
NPAD = NKT * KT
NCB = 4                     # nodeh column blocks (512 cols each)
CBC = 5                     # DMA chunks per column block (16 k-tiles, 2 MB each)
KCH = NKT // CBC            # 16 k-tiles per chunk

F32 = mybir.dt.float32
BF16 = mybir.dt.bfloat16
NPBF = ml_dtypes.bfloat16

SIG = mybir.ActivationFunctionType.Sigmoid
TANH = mybir.ActivationFunctionType.Tanh
IDENT = mybir.ActivationFunctionType.Identity

ADD = mybir.AluOpType.add
MUL = mybir.AluOpType.mult

# bf16 packed constants (matmul operands), column offsets
C_WC = 0
C_WTAU = 128
C_WX2 = 256
C_WRES = 384
C_WE2 = 512
C_WX1 = 640
C_WE1 = 641
C_W2 = 642            # [128, 7]
C_XIN = 649           # [128, 64]
C_T0 = 713
C_END = 777
C_TAU = 841           # [128, 1024]
C_COORDS = 1865       # [128, 2048]
CPW = 3920

# fp32 packed scalar biases, column offsets
Z_BTAU = 0
Z_BX2 = 1
Z_BRES = 2
Z_BE2 = 3
Z_B1 = 8              # [128, 7]
Z_B2 = 15             # [1, 7] (row 0)
CBW = 24

# Independent LSTM group chains, each prefix-ordered by descending T so the
# active set is always a prefix.  Gate block order [i, f, o, g].
GROUPS = [[0], [5], [4, 6], [2, 3, 1]]
TLEN = {0: 5 * L + 3, 1: L + 3, 2: 2 * L, 3: 2 * L, 4: 3 * L + 3,
        5: 4 * L, 6: 3 * L + 3}
GW = [len(g) * BL for g in GROUPS]          # 64, 64, 128, 192
TORCH = [0, 1, 3, 2]                        # block p -> torch gate index
NGK = sum(4 * len(g) for g in GROUPS)       # one-hot rows total (28)
BOH_OFF = []
_off = 0
for _g in GROUPS:
    BOH_OFF.append(_off)
    _off += 4 * len(_g) * BL
BOHW = _off                                  # 1792

_prog_cache = {}


def _build_program():
    """One SPMD Bass program; every core runs it on its own 64-row batch shard."""
    nc = bacc.Bacc()

    d_x = nc.declare_dram_parameter("xk", [NCB, CBC, 128, KCH, 512], BF16,
                                    isOutput=False)
    d_wn = nc.declare_dram_parameter("wn", [128, NKT, H], BF16, isOutput=False)
    d_cp = nc.declare_dram_parameter("cpack", [128, CPW], BF16, isOutput=False)
    d_cb = nc.declare_dram_parameter("cbias", [128, CBW], F32, isOutput=False)
    d_bt = nc.declare_dram_parameter("biasT", [NGK, 4 * 128], BF16, isOutput=False)
    d_oh = nc.declare_dram_parameter("onehot", [NGK, BOHW], BF16, isOutput=False)
    d_wih = nc.declare_dram_parameter("wihT", [H, 7, 4 * H], BF16, isOutput=False)
    d_whh = nc.declare_dram_parameter("whhT", [H, 7, 4 * H], BF16, isOutput=False)
    d_w1 = nc.declare_dram_parameter("w1T", [H, 7, HU], BF16, isOutput=False)
    d_out = nc.declare_dram_parameter("out", [1, 7 * BL], F32, isOutput=True)

    with tile.TileContext(nc) as tc:
        with (
            tc.tile_pool(name="consts", bufs=1) as consts,
            tc.tile_pool(name="xpool", bufs=3) as xpool,
            tc.tile_pool(name="gact", bufs=2) as gactp,
            tc.tile_pool(name="stp", bufs=2) as stp,
            tc.tile_pool(name="ps1", bufs=1, space="PSUM") as ps1,
            tc.tile_pool(name="gemm", bufs=2, space="PSUM") as gemmp,
            tc.tile_pool(name="g0", bufs=1, space="PSUM") as g0p,
            tc.tile_pool(name="g1", bufs=1, space="PSUM") as g1p,
            tc.tile_pool(name="g2", bufs=1, space="PSUM") as g2p,
            tc.tile_pool(name="g3", bufs=1, space="PSUM") as g3p,
        ):
            gpools = [g0p, g1p, g2p, g3p]

            cp = consts.tile([128, CPW], BF16, tag="cp")
            nc.sync.dma_start(cp[:], d_cp[:])
            cb = consts.tile([128, CBW], F32, tag="cb")
            nc.sync.dma_start(cb[:], d_cb[:])
            bt = consts.tile([NGK, 4 * 128], BF16, tag="bt")
            nc.sync.dma_start(bt[:], d_bt[:])
            oh = consts.tile([NGK, BOHW], BF16, tag="oh")
            nc.sync.dma_start(oh[:], d_oh[:])
            wih_sb = consts.tile([H, 7, 4 * H], BF16, tag="wih")
            nc.sync.dma_start(wih_sb[:], d_wih[:])
            whh_sb = consts.tile([H, 7, 4 * H], BF16, tag="whh")
            nc.sync.dma_start(whh_sb[:], d_whh[:])
            w1_sb = consts.tile([H, 7, HU], BF16, tag="w1")
            nc.sync.dma_start(w1_sb[:], d_w1[:])
            wn_sb = consts.tile([128, NKT, H], BF16, tag="wn")
            nc.sync.dma_start(wn_sb[:], d_wn[:])

            tauh_sb = consts.tile([H, LCOLS], BF16, tag="tauh")
            coordh_sb = consts.tile([H, COLS], BF16, tag="coordh")
            nodeh_cb = [consts.tile([H, 512], BF16, tag=f"nodeh{c}",
                                    name=f"nodeh{c}") for c in range(NCB)]
            xh_sb = consts.tile([H, BL], BF16, tag="xh")
            t0h_sb = consts.tile([H, BL], BF16, tag="t0h")
            endh_sb = consts.tile([H, BL], BF16, tag="endh")
            out_sb = consts.tile([1, 7 * BL], F32, tag="outsb")

            # persistent LSTM states per group (bf16, prefix column blocks)
            h_t = [consts.tile([H, w], BF16, tag=f"h{g}", name=f"h{g}")
                   for g, w in enumerate(GW)]
            c_t = [consts.tile([H, w], BF16, tag=f"c{g}", name=f"c{g}")
                   for g, w in enumerate(GW)]

            # ---- small projections ----
            for j in range(LCOLS // 512):
                ps = ps1.tile([128, 512], F32, tag="ps")
                nc.tensor.matmul(ps[:], cp[:, C_WTAU:C_WTAU + 128],
                                 cp[:, C_TAU + j * 512:C_TAU + (j + 1) * 512],
                                 start=True, stop=True)
                nc.scalar.activation(tauh_sb[:, j * 512:(j + 1) * 512], ps[:],
                                     TANH, bias=cb[:, Z_BTAU:Z_BTAU + 1])

            pst = ps1.tile([128, 512], F32, tag="ps", name="psx1")
            psa = pst[0:1, 0:BL]
            nc.tensor.matmul(psa[:], cp[:, C_WX1:C_WX1 + 1],
                             cp[:, C_XIN:C_XIN + BL], start=True, stop=True)
            s1_sb = stp.tile([128, BL], BF16, tag="svec")
            nc.vector.memset(s1_sb[:], 0.0)
            nc.vector.tensor_copy(s1_sb[0:1, :], psa[:])
            pst = ps1.tile([128, 512], F32, tag="ps", name="psx2")
            psb = pst[:, 0:BL]
            nc.tensor.matmul(psb[:], cp[:, C_WX2:C_WX2 + 128], s1_sb[:],
                             start=True, stop=True)
            nc.scalar.activation(xh_sb[:], psb[:], TANH,
                                 bias=cb[:, Z_BX2:Z_BX2 + 1])

            pst = ps1.tile([128, 512], F32, tag="ps", name="psx3")
            psc = pst[:, 0:BL]
            nc.tensor.matmul(psc[:], cp[:, C_WRES:C_WRES + 128],
                             cp[:, C_T0:C_T0 + BL], start=True, stop=True)
            nc.scalar.activation(t0h_sb[:], psc[:], TANH,
                                 bias=cb[:, Z_BRES:Z_BRES + 1])

            pst = ps1.tile([128, 512], F32, tag="ps", name="psx4")
            psd = pst[0:1, 0:BL]
            nc.tensor.matmul(psd[:], cp[:, C_WE1:C_WE1 + 1],
                             cp[:, C_END:C_END + BL], start=True, stop=True)
            s2_sb = stp.tile([128, BL], BF16, tag="svec")
            nc.vector.memset(s2_sb[:], 0.0)
            nc.vector.tensor_copy(s2_sb[0:1, :], psd[:])
            pst = ps1.tile([128, 512], F32, tag="ps", name="psx5")
            pse = pst[:, 0:BL]
            nc.tensor.matmul(pse[:], cp[:, C_WE2:C_WE2 + 128], s2_sb[:],
                             start=True, stop=True)
            nc.scalar.activation(endh_sb[:], pse[:], IDENT,
                                 bias=cb[:, Z_BE2:Z_BE2 + 1])

            for j in range(COLS // 512):
                ps = ps1.tile([128, 512], F32, tag="ps")
                nc.tensor.matmul(ps[:], cp[:, C_WC:C_WC + 128],
                                 cp[:, C_COORDS + j * 512:C_COORDS + (j + 1) * 512],
                                 start=True, stop=True)
                nc.vector.tensor_copy(coordh_sb[:, j * 512:(j + 1) * 512], ps[:])

            # ---- node GEMM: column-block streamed so nodeh lands early.
            # Block 0 runs upfront; blocks 1-3 are interleaved into the
            # superstep loop as dense PE filler (keeps HAM warm + paces DMA).
            gemm_state = {}

            def gemm_chunk(cbk, ch):
                if ch == 0:
                    gemm_state[cbk] = gemmp.tile([128, 512], F32, tag="gemmps",
                                                 name=f"gemmps{cbk}")
                gps = gemm_state[cbk]
                xt = xpool.tile([128, KCH, 512], BF16, tag="xt")
                nc.sync.dma_start(xt[:], d_x[cbk, ch])
                for kt in range(KCH):
                    kk = ch * KCH + kt
                    nc.tensor.matmul(gps[:], wn_sb[:, kk], xt[:, kt],
                                     start=(kk == 0), stop=(kk == NKT - 1))
                if ch == CBC - 1:
                    nc.vector.tensor_copy(nodeh_cb[cbk][:], gps[:])

            for ch in range(CBC):
                gemm_chunk(0, ch)
            # block c's last chunk lands at t = 8(c-1)+5, before the first
            # consumer (k2 reads block c at superstep 8c)
            gemm_sched = {}
            for cbk in range(1, NCB):
                for ch in range(CBC):
                    gemm_sched.setdefault(8 * (cbk - 1) + 1 + ch, []).append(
                        (cbk, ch))

            # ---- sequences ----
            def ncol(t):
                return nodeh_cb[t >> 3][:, (t & 7) * BL:((t & 7) + 1) * BL]

            def ccol(t):
                return coordh_sb[:, t * BL:(t + 1) * BL]

            def lcol(l):
                return tauh_sb[:, l * BL:(l + 1) * BL]

            seqs = {}
            pre = [lambda: xh_sb[:], lambda: t0h_sb[:]]
            suf = [lambda: endh_sb[:]]
            mk = lambda f, *a: (lambda: f(*a))
            seqs[0] = pre + [mk(f, t) for l in range(L)
                             for f, t in ((lcol, l), (ncol, 2 * l), (ccol, 2 * l),
                                          (ncol, 2 * l + 1), (ccol, 2 * l + 1))] + suf
            seqs[1] = pre + [mk(lcol, l) for l in range(L)] + suf
            seqs[2] = [mk(ncol, t) for t in range(T2)]
            seqs[3] = [mk(ccol, t) for t in range(T2)]
            seqs[4] = pre + [mk(f, t) for l in range(L)
                             for f, t in ((lcol, l), (ncol, 2 * l),
                                          (ncol, 2 * l + 1))] + suf
            seqs[5] = [mk(f, t) for l in range(L)
                       for f, t in ((ncol, 2 * l), (ccol, 2 * l),
                                    (ncol, 2 * l + 1), (ccol, 2 * l + 1))]
            seqs[6] = pre + [mk(f, t) for l in range(L)
                             for f, t in ((lcol, l), (ccol, 2 * l),
                                          (ccol, 2 * l + 1))] + suf

            def superstep(g, t):
                ks = GROUPS[g]
                W = GW[g]
                A = sum(1 for k in ks if t < TLEN[k])
                cols = A * BL
                nk = len(ks)
                gp = gpools[g].tile([128, 4, W], F32, tag="gp")
                flat = gp[:].rearrange("p a b -> p (a b)")
                # bias matmul(s) first: start=True covers each whole bank, so
                # all ih/hh matmuls accumulate with start=False afterwards.
                lhs = bt[0:4 * nk, g * 128:(g + 1) * 128]
                nbank = (4 * W + 511) // 512
                for b in range(nbank):
                    lo, hi = b * 512, min((b + 1) * 512, 4 * W)
                    nc.tensor.matmul(flat[:, lo:hi], lhs,
                                     oh[0:4 * nk, BOH_OFF[g] + lo:BOH_OFF[g] + hi],
                                     start=True, stop=False, skip_group_check=True)
                for p in range(4):
                    tg = TORCH[p]
                    gsl = slice(tg * H, (tg + 1) * H)
                    for j in range(A):
                        k = ks[j]
                        xc = seqs[k][t]()
                        dst = gp[:, p, j * BL:(j + 1) * BL]
                        last = (p == 3 and j == A - 1)
                        nc.tensor.matmul(dst, wih_sb[:, k, gsl], xc,
                                         start=False, stop=last and t == 0,
                                         skip_group_check=True)
                        if t > 0:
                            nc.tensor.matmul(dst, whh_sb[:, k, gsl],
                                             h_t[g][:, j * BL:(j + 1) * BL],
                                             start=False, stop=last,
                                             skip_group_check=True)
                ga = gactp.tile([128, 4, W], BF16, tag=f"gact{g}")
                nc.scalar.activation(ga[:, 0:3, :cols], gp[:, 0:3, :cols], SIG)
                nc.scalar.activation(ga[:, 3, :cols], gp[:, 3, :cols], TANH)
                if t == 0:
                    nc.vector.tensor_tensor(c_t[g][:, :cols], ga[:, 0, :cols],
                                            ga[:, 3, :cols], MUL)
                else:
                    ig = stp.tile([128, W], BF16, tag=f"ig{g}")
                    nc.vector.tensor_tensor(ig[:, :cols], ga[:, 0, :cols],
                                            ga[:, 3, :cols], MUL)
                    fc = stp.tile([128, W], BF16, tag=f"fc{g}")
                    nc.vector.tensor_tensor(fc[:, :cols], ga[:, 1, :cols],
                                            c_t[g][:, :cols], MUL)
                    nc.vector.tensor_tensor(c_t[g][:, :cols], fc[:, :cols],
                                            ig[:, :cols], ADD)
                tcc = stp.tile([128, W], BF16, tag=f"tc{g}")
                nc.scalar.activation(tcc[:, :cols], c_t[g][:, :cols], TANH)
                nc.vector.tensor_tensor(h_t[g][:, :cols], ga[:, 2, :cols],
                                        tcc[:, :cols], MUL)

            # interleave the group chains in program order per timestep
            tmax = max(TLEN[g[0]] for g in GROUPS)
            for t in range(tmax):
                for pair in gemm_sched.get(t, ()):
                    gemm_chunk(*pair)
                for g in range(len(GROUPS)):
                    if t < TLEN[GROUPS[g][0]]:
                        superstep(g, t)

            # ---- heads ----
            def head(k, h_ap, pool):
                hp_t = pool.tile([128, 512], F32, tag="gp", name=f"hp{k}")
                hp = hp_t[:, 0:BL]
                nc.tensor.matmul(hp[:], w1_sb[:, k], h_ap, start=True, stop=True)
                z1 = stp.tile([128, BL], BF16, tag="z1")
                nc.scalar.activation(z1[:], hp[:], TANH,
                                     bias=cb[:, Z_B1 + k:Z_B1 + k + 1])
                op_t = pool.tile([128, 512], F32, tag="gp", name=f"op{k}")
                op = op_t[0:1, 0:BL]
                nc.tensor.matmul(op[:], cp[:, C_W2 + k:C_W2 + k + 1], z1[:],
                                 start=True, stop=True)
                nc.scalar.activation(out_sb[:, k * BL:(k + 1) * BL], op[:],
                                     IDENT, bias=cb[0:1, Z_B2 + k:Z_B2 + k + 1])

            for g, ks in enumerate(GROUPS):
                for j, k in enumerate(ks):
                    head(k, h_t[g][:, j * BL:(j + 1) * BL], gpools[g])

            nc.sync.dma_start(d_out[:], out_sb[:])

    nc.finalize()
    return nc


def _get_program():
    if "nc" not in _prog_cache:
        _prog_cache["nc"] = _build_program()
    return _prog_cache["nc"]


def _pack_constants(inp):
    cpk = np.zeros((128, CPW), NPBF)
    cbk = np.zeros((128, CBW), np.float32)

    def put(dst, c, arr):
        dst[:arr.shape[0], c:c + arr.shape[1]] = arr

    put(cpk, C_WC, inp["Wcoord"].T)
    put(cpk, C_WTAU, inp["Wtau"].T)
    put(cpk, C_WX2, inp["Wx2"].T)
    put(cpk, C_WRES, inp["Wres"].T)
    put(cpk, C_WE2, inp["Wend2"].T)
    put(cpk, C_WX1, inp["Wx1"].T)
    put(cpk, C_WE1, inp["Wend1"].T)
    put(cpk, C_W2, inp["head_W2"].reshape(7, HU).T)
    put(cbk, Z_BTAU, inp["btau"][:, None])
    put(cbk, Z_BX2, inp["bx2"][:, None])
    put(cbk, Z_BRES, inp["bres"][:, None])
    put(cbk, Z_BE2, inp["bend2"][:, None])
    put(cbk, Z_B1, inp["head_b1"].T)
    put(cbk, Z_B2, inp["head_b2"].reshape(1, 7))

    # bias-matmul constants: lhsT rows r = p*nk + j hold bsum[k_j, torch(p)];
    # one-hot rhs puts that row's 128-vector into block column (p, j)
    bsum = (inp["lstm_bih"] + inp["lstm_bhh"]).reshape(7, 4, H)
    btk = np.zeros((NGK, 4 * 128), np.float32)
    ohk = np.zeros((NGK, BOHW), NPBF)
    for g, ks in enumerate(GROUPS):
        nk = len(ks)
        for p in range(4):
            tg = TORCH[p]
            for j, k in enumerate(ks):
                r = p * nk + j
                btk[r, g * 128:(g + 1) * 128] = bsum[k, tg]
                base = BOH_OFF[g] + (p * nk + j) * BL
                ohk[r, base:base + BL] = 1.0
    return cpk, cbk, btk.astype(NPBF), ohk


def _make_in_maps(inp):
    node = inp["node_inputs"]
    coords = inp["coords"]
    tau = inp["tau_inputs"]
    x = inp["x"]
    t0 = inp["t0_res"]
    end = inp["end"]

    wn = np.zeros((NPAD, H), NPBF)
    wn[:N] = inp["Wnode"].T
    wn_dev = np.ascontiguousarray(wn.reshape(NKT, 128, H).transpose(1, 0, 2))

    wih = np.ascontiguousarray(inp["lstm_Wih"].transpose(2, 0, 1).astype(NPBF))
    whh = np.ascontiguousarray(inp["lstm_Whh"].transpose(2, 0, 1).astype(NPBF))
    w1 = np.ascontiguousarray(inp["head_W1"].transpose(2, 0, 1).astype(NPBF))

    cpk_base, cbk, btk, ohk = _pack_constants(inp)

    in_maps = []
    for c in range(NCORES):
        sl = slice(c * BL, (c + 1) * BL)
        xk = np.zeros((NPAD, COLS), NPBF)
        xk[:N] = node[sl].transpose(2, 1, 0).reshape(N, COLS)
        # [NCB, CBC, 128, KCH, 512]: col-block-major streaming layout
        xk_dev = np.ascontiguousarray(
            xk.reshape(CBC, KCH, 128, NCB, 512).transpose(3, 0, 2, 1, 4))
        cpk = cpk_base.copy()
        cpk[:2, C_XIN:C_XIN + BL] = x[sl].T
        cpk[:1, C_T0:C_T0 + BL] = t0[sl].T
        cpk[:2, C_END:C_END + BL] = end[sl].T
        cpk[:1, C_TAU:C_TAU + LCOLS] = tau[sl].transpose(2, 1, 0).reshape(1, LCOLS)
        cpk[:2, C_COORDS:C_COORDS + COLS] = coords[sl].transpose(2, 1, 0).reshape(2, COLS)
        in_maps.append(dict(
            xk=xk_dev, wn=wn_dev, cpack=cpk, cbias=cbk,
            biasT=btk, onehot=ohk, wihT=wih, whhT=whh, w1T=w1,
        ))
    return in_maps


def kernel(**inputs):
    inp = {k: np.asarray(v, dtype=np.float32) for k, v in inputs.items()}
    in_maps = _make_in_maps(inp)
    nc = _get_program()
    res = run_bass_kernel_spmd(nc, in_maps, core_ids=list(range(NCORES)))
    if res.exec_time_ns is not None:
        print(f"HW exec time: {res.exec_time_ns} ns")

    outs = [r["out"].reshape(7, BL) for r in res.results]
    full = np.concatenate(outs, axis=1)      # [7, B]
    return tuple(full[k][:, None].astype(np.float32) for k in range(7))


# revision 11
# speedup vs baseline: 1.7915x; 1.0942x over previous
import os
import sys

import numpy as np

sys.path.insert(0, "/opt/trn_rl_repo")

import ml_dtypes
import concourse.bass as bass
from concourse import bacc
import concourse.mybir as mybir
import concourse.tile as tile
from concourse.bass_utils import run_bass_kernel_spmd

# Problem constants (hardcoded per contract)
B, L, N, H, HU = 512, 16, 10000, 128, 128
NCORES = 8
BL = B // NCORES            # 64 local batch rows per core
T2 = 2 * L                  # 32 node/coord time steps
COLS = T2 * BL              # 2048 node/coord columns, t-major: col = t*BL + b
LCOLS = L * BL              # 1024 tau columns
KT = 128
NKT = 80                    # k-tiles (padded: 80*128 = 10240 >= 10000)
NPAD = NKT * KT
NCB = 4                     # nodeh column blocks (512 cols each)
CBC = 5                     # DMA chunks per column block (16 k-tiles, 2 MB each)
KCH = NKT // CBC            # 16 k-tiles per chunk

F32 = mybir.dt.float32
BF16 = mybir.dt.bfloat16
NPBF = ml_dtypes.bfloat16

SIG = mybir.ActivationFunctionType.Sigmoid
TANH = mybir.ActivationFunctionType.Tanh
IDENT = mybir.ActivationFunctionType.Identity

ADD = mybir.AluOpType.add
MUL = mybir.AluOpType.mult

# bf16 packed constants (matmul operands), column offsets
C_WC = 0
C_WTAU = 128
C_WX2 = 256
C_WRES = 384
C_WE2 = 512
C_WX1 = 640
C_WE1 = 641
C_W2 = 642            # [128, 7]
C_XIN = 649           # [128, 64]
C_T0 = 713
C_END = 777
C_TAU = 841           # [128, 1024]
C_COORDS = 1865       # [128, 2048]
CPW = 3920

# fp32 packed scalar biases, column offsets
Z_BTAU = 0
Z_BX2 = 1
Z_BRES = 2
Z_BE2 = 3
Z_B1 = 8              # [128, 7]
Z_B2 = 15             # [1, 7] (row 0)
CBW = 24

# Independent LSTM group chains, each prefix-ordered by descending T so the
# active set is always a prefix.  Gate block order [i, f, o, g].
GROUPS = [[0], [5], [4, 6], [2, 3, 1]]
TLEN = {0: 5 * L + 3, 1: L + 3, 2: 2 * L, 3: 2 * L, 4: 3 * L + 3,
        5: 4 * L, 6: 3 * L + 3}
GW = [len(g) * BL for g in GROUPS]          # 64, 64, 128, 192
TORCH = [0, 1, 3, 2]                        # block p -> torch gate index
NGK = sum(4 * len(g) for g in GROUPS)       # one-hot rows total (28)
BOH_OFF = []
_off = 0
for _g in GROUPS:
    BOH_OFF.append(_off)
    _off += 4 * len(_g) * BL
BOHW = _off                                  # 1792

_prog_cache = {}


def _build_program():
    """One SPMD Bass program; every core runs it on its own 64-row batch shard."""
    nc = bacc.Bacc()

    d_x = nc.declare_dram_parameter("xk", [NCB, CBC, 128, KCH, 512], BF16,
                                    isOutput=False)
    d_wn = nc.declare_dram_parameter("wn", [128, NKT, H], BF16, isOutput=False)
    d_cp = nc.declare_dram_parameter("cpack", [128, CPW], BF16, isOutput=False)
    d_cb = nc.declare_dram_parameter("cbias", [128, CBW], F32, isOutput=False)
    d_bt = nc.declare_dram_parameter("biasT", [NGK, 4 * 128], BF16, isOutput=False)
    d_oh = nc.declare_dram_parameter("onehot", [NGK, BOHW], BF16, isOutput=False)
    d_wih = nc.declare_dram_parameter("wihT", [H, 7, 4 * H], BF16, isOutput=False)
    d_whh = nc.declare_dram_parameter("whhT", [H, 7, 4 * H], BF16, isOutput=False)
    d_w1 = nc.declare_dram_parameter("w1T", [H, 7, HU], BF16, isOutput=False)
    d_out = nc.declare_dram_parameter("out", [1, 7 * BL], F32, isOutput=True)

    with tile.TileContext(nc) as tc:
        with (
            tc.tile_pool(name="consts", bufs=1) as consts,
            tc.tile_pool(name="xpool", bufs=3) as xpool,
            tc.tile_pool(name="gact", bufs=2) as gactp,
            tc.tile_pool(name="stp", bufs=2) as stp,
            tc.tile_pool(name="ps1", bufs=1, space="PSUM") as ps1,
            tc.tile_pool(name="gemm", bufs=2, space="PSUM") as gemmp,
            tc.tile_pool(name="g0", bufs=1, space="PSUM") as g0p,
            tc.tile_pool(name="g1", bufs=1, space="PSUM") as g1p,
            tc.tile_pool(name="g2", bufs=1, space="PSUM") as g2p,
            tc.tile_pool(name="g3", bufs=1, space="PSUM") as g3p,
        ):
            gpools = [g0p, g1p, g2p, g3p]

            cp = consts.tile([128, CPW], BF16, tag="cp")
            nc.sync.dma_start(cp[:], d_cp[:])
            cb = consts.tile([128, CBW], F32, tag="cb")
            nc.sync.dma_start(cb[:], d_cb[:])
            bt = consts.tile([NGK, 4 * 128], BF16, tag="bt")
            nc.sync.dma_start(bt[:], d_bt[:])
            oh = consts.tile([NGK, BOHW], BF16, tag="oh")
            nc.sync.dma_start(oh[:], d_oh[:])
            wih_sb = consts.tile([H, 7, 4 * H], BF16, tag="wih")
            nc.sync.dma_start(wih_sb[:], d_wih[:])
            whh_sb = consts.tile([H, 7, 4 * H], BF16, tag="whh")
            nc.sync.dma_start(whh_sb[:], d_whh[:])
            w1_sb = consts.tile([H, 7, HU], BF16, tag="w1")
            nc.sync.dma_start(w1_sb[:], d_w1[:])
            wn_sb = consts.tile([128, NKT, H], BF16, tag="wn")
            nc.sync.dma_start(wn_sb[:], d_wn[:])

            tauh_sb = consts.tile([H, LCOLS], BF16, tag="tauh")
            coordh_sb = consts.tile([H, COLS], BF16, tag="coordh")
            nodeh_cb = [consts.tile([H, 512], BF16, tag=f"nodeh{c}",
                                    name=f"nodeh{c}") for c in range(NCB)]
            xh_sb = consts.tile([H, BL], BF16, tag="xh")
            t0h_sb = consts.tile([H, BL], BF16, tag="t0h")
            endh_sb = consts.tile([H, BL], BF16, tag="endh")
            out_sb = consts.tile([1, 7 * BL], F32, tag="outsb")

            # persistent LSTM states per group (bf16, prefix column blocks)
            h_t = [consts.tile([H, w], BF16, tag=f"h{g}", name=f"h{g}")
                   for g, w in enumerate(GW)]
            c_t = [consts.tile([H, w], BF16, tag=f"c{g}", name=f"c{g}")
                   for g, w in enumerate(GW)]

            # ---- small projections ----
            for j in range(LCOLS // 512):
                ps = ps1.tile([128, 512], F32, tag="ps")
                nc.tensor.matmul(ps[:], cp[:, C_WTAU:C_WTAU + 128],
                                 cp[:, C_TAU + j * 512:C_TAU + (j + 1) * 512],
                                 start=True, stop=True)
                nc.scalar.activation(tauh_sb[:, j * 512:(j + 1) * 512], ps[:],
                                     TANH, bias=cb[:, Z_BTAU:Z_BTAU + 1])

            pst = ps1.tile([128, 512], F32, tag="ps", name="psx1")
            psa = pst[0:1, 0:BL]
            nc.tensor.matmul(psa[:], cp[:, C_WX1:C_WX1 + 1],
                             cp[:, C_XIN:C_XIN + BL], start=True, stop=True)
            s1_sb = stp.tile([128, BL], BF16, tag="svec")
            nc.vector.memset(s1_sb[:], 0.0)
            nc.vector.tensor_copy(s1_sb[0:1, :], psa[:])
            pst = ps1.tile([128, 512], F32, tag="ps", name="psx2")
            psb = pst[:, 0:BL]
            nc.tensor.matmul(psb[:], cp[:, C_WX2:C_WX2 + 128], s1_sb[:],
                             start=True, stop=True)
            nc.scalar.activation(xh_sb[:], psb[:], TANH,
                                 bias=cb[:, Z_BX2:Z_BX2 + 1])

            pst = ps1.tile([128, 512], F32, tag="ps", name="psx3")
            psc = pst[:, 0:BL]
            nc.tensor.matmul(psc[:], cp[:, C_WRES:C_WRES + 128],
                             cp[:, C_T0:C_T0 + BL], start=True, stop=True)
            nc.scalar.activation(t0h_sb[:], psc[:], TANH,
                                 bias=cb[:, Z_BRES:Z_BRES + 1])

            pst = ps1.tile([128, 512], F32, tag="ps", name="psx4")
            psd = pst[0:1, 0:BL]
            nc.tensor.matmul(psd[:], cp[:, C_WE1:C_WE1 + 1],
                             cp[:, C_END:C_END + BL], start=True, stop=True)
            s2_sb = stp.tile([128, BL], BF16, tag="svec")
            nc.vector.memset(s2_sb[:], 0.0)
            nc.vector.tensor_copy(s2_sb[0:1, :], psd[:])
            pst = ps1.tile([128, 512], F32, tag="ps", name="psx5")
            pse = pst[:, 0:BL]
            nc.tensor.matmul(pse[:], cp[:, C_WE2:C_WE2 + 128], s2_sb[:],
                             start=True, stop=True)
            nc.scalar.activation(endh_sb[:], pse[:], IDENT,
                                 bias=cb[:, Z_BE2:Z_BE2 + 1])

            for j in range(COLS // 512):
                ps = ps1.tile([128, 512], F32, tag="ps")
                nc.tensor.matmul(ps[:], cp[:, C_WC:C_WC + 128],
                                 cp[:, C_COORDS + j * 512:C_COORDS + (j + 1) * 512],
                                 start=True, stop=True)
                nc.vector.tensor_copy(coordh_sb[:, j * 512:(j + 1) * 512], ps[:])

            # ---- node GEMM: column-block streamed so nodeh lands early.
            # Block 0 runs upfront; blocks 1-3 are interleaved into the
            # superstep loop as dense PE filler (keeps HAM warm + paces DMA).
            gemm_state = {}

            def gemm_chunk(cbk, ch):
                if ch == 0:
                    gemm_state[cbk] = gemmp.tile([128, 512], F32, tag="gemmps",
                                                 name=f"gemmps{cbk}")
                gps = gemm_state[cbk]
                xt = xpool.tile([128, KCH, 512], BF16, tag="xt")
                nc.sync.dma_start(xt[:], d_x[cbk, ch])
                for kt in range(KCH):
                    kk = ch * KCH + kt
                    nc.tensor.matmul(gps[:], wn_sb[:, kk], xt[:, kt],
                                     start=(kk == 0), stop=(kk == NKT - 1))
                if ch == CBC - 1:
                    nc.vector.tensor_copy(nodeh_cb[cbk][:], gps[:])

            for ch in range(CBC):
                gemm_chunk(0, ch)
            # Spread blocks 1-3 across the whole loop (PE filler keeps the HAM
            # clock at 8/8); groups stall on block availability as needed.
            gemm_sched = {}
            _rest = [(cbk, ch) for cbk in range(1, NCB) for ch in range(CBC)]
            for i, pair in enumerate(_rest):
                gemm_sched.setdefault(1 + 4 * i, []).append(pair)
            _blk_done = [0]                       # round when block is complete
            for i in range(len(_rest)):
                if _rest[i][1] == CBC - 1:
                    _blk_done.append(1 + 4 * i)

            # ---- sequences ----
            def ncol(t):
                return nodeh_cb[t >> 3][:, (t & 7) * BL:((t & 7) + 1) * BL]

            def ccol(t):
                return coordh_sb[:, t * BL:(t + 1) * BL]

            def lcol(l):
                return tauh_sb[:, l * BL:(l + 1) * BL]

            seqs = {}
            nreq = {}     # nreq[k][t] = node block index used at step t (-1 none)
            pre = [lambda: xh_sb[:], lambda: t0h_sb[:]]
            suf = [lambda: endh_sb[:]]
            mk = lambda f, *a: (lambda: f(*a))
            seqs[0] = pre + [mk(f, t) for l in range(L)
                             for f, t in ((lcol, l), (ncol, 2 * l), (ccol, 2 * l),
                                          (ncol, 2 * l + 1), (ccol, 2 * l + 1))] + suf
            seqs[1] = pre + [mk(lcol, l) for l in range(L)] + suf
            seqs[2] = [mk(ncol, t) for t in range(T2)]
            seqs[3] = [mk(ccol, t) for t in range(T2)]
            seqs[4] = pre + [mk(f, t) for l in range(L)
                             for f, t in ((lcol, l), (ncol, 2 * l),
                                          (ncol, 2 * l + 1))] + suf
            seqs[5] = [mk(f, t) for l in range(L)
                       for f, t in ((ncol, 2 * l), (ccol, 2 * l),
                                    (ncol, 2 * l + 1), (ccol, 2 * l + 1))]
            seqs[6] = pre + [mk(f, t) for l in range(L)
                             for f, t in ((lcol, l), (ccol, 2 * l),
                                          (ccol, 2 * l + 1))] + suf

            _pats = {0: [(0, None), (1, None)] +
                        [(2, v) for l in range(L)
                         for v in (None, 2 * l, None, 2 * l + 1, None)] + [(3, None)],
                     1: None, 3: None, 6: None,
                     2: [(2, t) for t in range(T2)],
                     4: [(0, None), (1, None)] +
                        [(2, v) for l in range(L)
                         for v in (None, 2 * l, 2 * l + 1)] + [(3, None)],
                     5: [(2, v) for l in range(L)
                         for v in (2 * l, None, 2 * l + 1, None)]}
            for k in range(7):
                pat = _pats[k]
                if pat is None:
                    nreq[k] = [-1] * TLEN[k]
                else:
                    nreq[k] = [(-1 if v is None else v >> 3) for _, v in pat]
                assert len(nreq[k]) == TLEN[k], (k, len(nreq[k]))

            def superstep(g, t):
                ks = GROUPS[g]
                W = GW[g]
                A = sum(1 for k in ks if t < TLEN[k])
                cols = A * BL
                nk = len(ks)
                gp = gpools[g].tile([128, 4, W], F32, tag="gp")
                flat = gp[:].rearrange("p a b -> p (a b)")
                # bias matmul(s) first: start=True covers each whole bank, so
                # all ih/hh matmuls accumulate with start=False afterwards.
                lhs = bt[0:4 * nk, g * 128:(g + 1) * 128]
                nbank = (4 * W + 511) // 512
                for b in range(nbank):
                    lo, hi = b * 512, min((b + 1) * 512, 4 * W)
                    nc.tensor.matmul(flat[:, lo:hi], lhs,
                                     oh[0:4 * nk, BOH_OFF[g] + lo:BOH_OFF[g] + hi],
                                     start=True, stop=False, skip_group_check=True)
                for p in range(4):
                    tg = TORCH[p]
                    gsl = slice(tg * H, (tg + 1) * H)
                    for j in range(A):
                        k = ks[j]
                        xc = seqs[k][t]()
                        dst = gp[:, p, j * BL:(j + 1) * BL]
                        last = (p == 3 and j == A - 1)
                        nc.tensor.matmul(dst, wih_sb[:, k, gsl], xc,
                                         start=False, stop=last and t == 0,
                                         skip_group_check=True)
                        if t > 0:
                            nc.tensor.matmul(dst, whh_sb[:, k, gsl],
                                             h_t[g][:, j * BL:(j + 1) * BL],
                                             start=False, stop=last,
                                             skip_group_check=True)
                ga = gactp.tile([128, 4, W], BF16, tag=f"gact{g}")
                nc.scalar.activation(ga[:, 0:3, :cols], gp[:, 0:3, :cols], SIG)
                nc.scalar.activation(ga[:, 3, :cols], gp[:, 3, :cols], TANH)
                if t == 0:
                    nc.vector.tensor_tensor(c_t[g][:, :cols], ga[:, 0, :cols],
                                            ga[:, 3, :cols], MUL)
                else:
                    ig = stp.tile([128, W], BF16, tag=f"ig{g}")
                    nc.vector.tensor_tensor(ig[:, :cols], ga[:, 0, :cols],
                                            ga[:, 3, :cols], MUL)
                    fc = stp.tile([128, W], BF16, tag=f"fc{g}")
                    nc.vector.tensor_tensor(fc[:, :cols], ga[:, 1, :cols],
                                            c_t[g][:, :cols], MUL)
                    nc.vector.tensor_tensor(c_t[g][:, :cols], fc[:, :cols],
                                            ig[:, :cols], ADD)
                tcc = stp.tile([128, W], BF16, tag=f"tc{g}")
                nc.scalar.activation(tcc[:, :cols], c_t[g][:, :cols], TANH)
                nc.vector.tensor_tensor(h_t[g][:, :cols], ga[:, 2, :cols],
                                        tcc[:, :cols], MUL)

            # Round loop: one superstep per group per round, gated on the node
            # block its inputs need having been emitted already (program order
            # must respect data flow; the scheduler handles the timing).
            tg = [0] * len(GROUPS)
            rounds = 0
            while any(tg[g] < TLEN[GROUPS[g][0]] for g in range(len(GROUPS))):
                for pair in gemm_sched.get(rounds, ()):
                    gemm_chunk(*pair)
                for g in range(len(GROUPS)):
                    t = tg[g]
                    if t >= TLEN[GROUPS[g][0]]:
                        continue
                    need = max(nreq[k][t] for k in GROUPS[g] if t < TLEN[k])
                    if need >= 0 and _blk_done[need] > rounds:
                        continue
                    superstep(g, t)
                    tg[g] += 1
                rounds += 1
            for pair_list in [v for r, v in gemm_sched.items() if r >= rounds]:
                for pair in pair_list:
                    gemm_chunk(*pair)

            # ---- heads ----
            def head(k, h_ap, pool):
                hp_t = pool.tile([128, 512], F32, tag="gp", name=f"hp{k}")
                hp = hp_t[:, 0:BL]
                nc.tensor.matmul(hp[:], w1_sb[:, k], h_ap, start=True, stop=True)
                z1 = stp.tile([128, BL], BF16, tag="z1")
                nc.scalar.activation(z1[:], hp[:], TANH,
                                     bias=cb[:, Z_B1 + k:Z_B1 + k + 1])
                op_t = pool.tile([128, 512], F32, tag="gp", name=f"op{k}")
                op = op_t[0:1, 0:BL]
                nc.tensor.matmul(op[:], cp[:, C_W2 + k:C_W2 + k + 1], z1[:],
                                 start=True, stop=True)
                nc.scalar.activation(out_sb[:, k * BL:(k + 1) * BL], op[:],
                                     IDENT, bias=cb[0:1, Z_B2 + k:Z_B2 + k + 1])

            for g, ks in enumerate(GROUPS):
                for j, k in enumerate(ks):
                    head(k, h_t[g][:, j * BL:(j + 1) * BL], gpools[g])

            nc.sync.dma_start(d_out[:], out_sb[:])

    nc.finalize()
    return nc


def _get_program():
    if "nc" not in _prog_cache:
        _prog_cache["nc"] = _build_program()
    return _prog_cache["nc"]


def _pack_constants(inp):
    cpk = np.zeros((128, CPW), NPBF)
    cbk = np.zeros((128, CBW), np.float32)

    def put(dst, c, arr):
        dst[:arr.shape[0], c:c + arr.shape[1]] = arr

    put(cpk, C_WC, inp["Wcoord"].T)
    put(cpk, C_WTAU, inp["Wtau"].T)
    put(cpk, C_WX2, inp["Wx2"].T)
    put(cpk, C_WRES, inp["Wres"].T)
    put(cpk, C_WE2, inp["Wend2"].T)
    put(cpk, C_WX1, inp["Wx1"].T)
    put(cpk, C_WE1, inp["Wend1"].T)
    put(cpk, C_W2, inp["head_W2"].reshape(7, HU).T)
    put(cbk, Z_BTAU, inp["btau"][:, None])
    put(cbk, Z_BX2, inp["bx2"][:, None])
    put(cbk, Z_BRES, inp["bres"][:, None])
    put(cbk, Z_BE2, inp["bend2"][:, None])
    put(cbk, Z_B1, inp["head_b1"].T)
    put(cbk, Z_B2, inp["head_b2"].reshape(1, 7))

    # bias-matmul constants: lhsT rows r = p*nk + j hold bsum[k_j, torch(p)];
    # one-hot rhs puts that row's 128-vector into block column (p, j)
    bsum = (inp["lstm_bih"] + inp["lstm_bhh"]).reshape(7, 4, H)
    btk = np.zeros((NGK, 4 * 128), np.float32)
    ohk = np.zeros((NGK, BOHW), NPBF)
    for g, ks in enumerate(GROUPS):
        nk = len(ks)
        for p in range(4):
            tg = TORCH[p]
            for j, k in enumerate(ks):
                r = p * nk + j
                btk[r, g * 128:(g + 1) * 128] = bsum[k, tg]
                base = BOH_OFF[g] + (p * nk + j) * BL
                ohk[r, base:base + BL] = 1.0
    return cpk, cbk, btk.astype(NPBF), ohk


def _make_in_maps(inp):
    node = inp["node_inputs"]
    coords = inp["coords"]
    tau = inp["tau_inputs"]
    x = inp["x"]
    t0 = inp["t0_res"]
    end = inp["end"]

    wn = np.zeros((NPAD, H), NPBF)
    wn[:N] = inp["Wnode"].T
    wn_dev = np.ascontiguousarray(wn.reshape(NKT, 128, H).transpose(1, 0, 2))

    wih = np.ascontiguousarray(inp["lstm_Wih"].transpose(2, 0, 1).astype(NPBF))
    whh = np.ascontiguousarray(inp["lstm_Whh"].transpose(2, 0, 1).astype(NPBF))
    w1 = np.ascontiguousarray(inp["head_W1"].transpose(2, 0, 1).astype(NPBF))

    cpk_base, cbk, btk, ohk = _pack_constants(inp)

    in_maps = []
    for c in range(NCORES):
        sl = slice(c * BL, (c + 1) * BL)
        xk = np.zeros((NPAD, COLS), NPBF)
        xk[:N] = node[sl].transpose(2, 1, 0).reshape(N, COLS)
        # [NCB, CBC, 128, KCH, 512]: col-block-major streaming layout
        xk_dev = np.ascontiguousarray(
            xk.reshape(CBC, KCH, 128, NCB, 512).transpose(3, 0, 2, 1, 4))
        cpk = cpk_base.copy()
        cpk[:2, C_XIN:C_XIN + BL] = x[sl].T
        cpk[:1, C_T0:C_T0 + BL] = t0[sl].T
        cpk[:2, C_END:C_END + BL] = end[sl].T
        cpk[:1, C_TAU:C_TAU + LCOLS] = tau[sl].transpose(2, 1, 0).reshape(1, LCOLS)
        cpk[:2, C_COORDS:C_COORDS + COLS] = coords[sl].transpose(2, 1, 0).reshape(2, COLS)
        in_maps.append(dict(
            xk=xk_dev, wn=wn_dev, cpack=cpk, cbias=cbk,
            biasT=btk, onehot=ohk, wihT=wih, whhT=whh, w1T=w1,
        ))
    return in_maps


def kernel(**inputs):
    inp = {k: np.asarray(v, dtype=np.float32) for k, v in inputs.items()}
    in_maps = _make_in_maps(inp)
    nc = _get_program()
    res = run_bass_kernel_spmd(nc, in_maps, core_ids=list(range(NCORES)))
    if res.exec_time_ns is not None:
        print(f"HW exec time: {res.exec_time_ns} ns")

    outs = [r["out"].reshape(7, BL) for r in res.results]
    full = np.concatenate(outs, axis=1)      # [7, B]
    return tuple(full[k][:, None].astype(np.float32) for k in range(7))
